# revision 14
# baseline (speedup 1.0000x reference)
"""Trainium2 Bass kernel for Llama GQA attention (B=2, S=2048, H=4096,
32 Q heads / 8 KV heads, head_dim 128, RoPE, causal).

Sharding: tensor-parallel by head across 8 cores. Core c owns Q heads
[4c..4c+3] and KV head c. Each core computes its Q/K/V projections,
RoPE, causal attention, and a partial output projection over its 512
attention features; the host sums the 8 partial outputs.

Device layout is feature-major ([feature, token]) throughout:
  - QKV proj:  Q'[f,t] (psum) = sum_h WqT[h,f].T @ xT[h,t]     (bf16)
  - RoPE:      q*cos + swap_halves(q)*sign*sin  (DVE + DMA swap)
  - scores:    S.T[k,q] = K'[d,k].T @ Q'[d,q]   (softmax over partition)
  - softmax:   exp on ACT (no max subtraction; scores are O(10)),
               denominator via ones-column matmul, fast reciprocal,
               K=1 broadcast matmul, normalize fused into psum evict
  - AV:        U[d,q] = Vtok[k,d].T @ E[k,q]    (bf16, causal-sliced)
  - out:       out[t,o] = attn'[f,t].T @ WoT[f,o]  (partial; host sums)

Batch-0's output projection is interleaved into batch-1's attention so
the PE chews o-proj matmuls while ACT runs the exp stream.
"""
import math
import numpy as np
import ml_dtypes

import concourse.bacc as bacc
import concourse.tile as tile
from concourse import mybir
from concourse.bass_utils import run_bass_kernel_spmd

F32 = mybir.dt.float32
F32R = mybir.dt.float32r
BF16 = mybir.dt.bfloat16

DT_PROJ = BF16
DT_QK = BF16
DT_ATT = BF16
NP_PROJ = ml_dtypes.bfloat16 if DT_PROJ == BF16 else np.float32
NP_ATT = ml_dtypes.bfloat16 if DT_ATT == BF16 else np.float32

P = 128
B, S, H = 2, 2048, 4096
T = B * S
DK = 128
NHL = 4
FL = NHL * DK
TB = 512
NTB = T // TB
NA = H // P
QBS = 512
NQB = S // QBS
NKT = S // P
SCALE = 1.0 / math.sqrt(DK)
NOB = H // 512
NTPB = S // P                # 16 output row tiles per batch

_NC_CACHE = {}


def build():
    nc = bacc.Bacc(None, target_bir_lowering=False)

    xt = nc.dram_tensor("xt", [H, T], DT_PROJ, kind="ExternalInput")
    wqt = nc.dram_tensor("wqt", [H, FL], DT_PROJ, kind="ExternalInput")
    wkt = nc.dram_tensor("wkt", [H, DK], DT_PROJ, kind="ExternalInput")
    wvt = nc.dram_tensor("wvt", [H, DK], DT_PROJ, kind="ExternalInput")
    wot = nc.dram_tensor("wot", [FL, H], DT_ATT, kind="ExternalInput")
    cost = nc.dram_tensor("cost", [P, S], F32, kind="ExternalInput")
    sints = nc.dram_tensor("sints", [P, S], F32, kind="ExternalInput")
    trimask = nc.dram_tensor("trimask", [P, P], BF16, kind="ExternalInput")
    onesc = nc.dram_tensor("onesc", [P, 1], BF16, kind="ExternalInput")
    out = nc.dram_tensor("out", [T, H], F32, kind="ExternalOutput")

    EXP = mybir.ActivationFunctionType.Exp

    with nc.allow_low_precision(reason="attention compute dtypes are "
                                       "deliberately reduced"), \
         tile.TileContext(nc) as tc:
        with tc.tile_pool(name="const", bufs=1) as cp, \
             tc.tile_pool(name="dram", bufs=1, space="DRAM") as dp, \
             tc.tile_pool(name="attn", bufs=1) as ap, \
             tc.tile_pool(name="p2", bufs=1) as p2, \
             tc.tile_pool(name="p2q", bufs=2) as p2q, \
             tc.tile_pool(name="p2e", bufs=5) as p2e, \
             tc.tile_pool(name="p3w", bufs=2) as p3w, \
             tc.tile_pool(name="p3o", bufs=4) as p3o:
            cos_sb = cp.tile([P, S], F32)
            sin_sb = cp.tile([P, S], F32)
            tri_sb = cp.tile([P, P], BF16)
            oc_sb = cp.tile([P, 1], BF16)
            nc.sync.dma_start(out=cos_sb, in_=cost[:, :])
            nc.sync.dma_start(out=sin_sb, in_=sints[:, :])
            nc.sync.dma_start(out=tri_sb, in_=trimask[:, :])
            nc.sync.dma_start(out=oc_sb, in_=onesc[:, :])

            attn_sb = [[ap.tile([P, S], DT_ATT, name=f"attn{b}_{h}")
                        for h in range(NHL)] for b in range(B)]
            q_scr = [dp.tile([FL, S], DT_QK, name=f"qscr{b}") for b in range(B)]
            k_scr = [dp.tile([DK, S], DT_QK, name=f"kscr{b}") for b in range(B)]
            v_scr = [dp.tile([DK, S], BF16, name=f"vscr{b}") for b in range(B)]

            # ---------------- Phase 1: QKV projection + RoPE ----------------
            with tc.tile_pool(name="wq", bufs=1) as wqp, \
                 tc.tile_pool(name="xp", bufs=6) as xp, \
                 tc.tile_pool(name="rp", bufs=1) as rp, \
                 tc.tile_pool(name="ps1", bufs=1, space="PSUM") as ps1:
                wq_sb = wqp.tile([P, NA * FL], DT_PROJ)
                wk_sb = wqp.tile([P, NA * DK], DT_PROJ)
                wv_sb = wqp.tile([P, NA * DK], DT_PROJ)
                for a in range(NA):
                    nc.sync.dma_start(out=wq_sb[:, a * FL:(a + 1) * FL],
                                      in_=wqt[a * P:(a + 1) * P, :])
                    nc.sync.dma_start(out=wk_sb[:, a * DK:(a + 1) * DK],
                                      in_=wkt[a * P:(a + 1) * P, :])
                    nc.sync.dma_start(out=wv_sb[:, a * DK:(a + 1) * DK],
                                      in_=wvt[a * P:(a + 1) * P, :])

                for tb in range(NTB):
                    bi = (tb * TB) // S
                    s0 = (tb * TB) % S
                    psq = [ps1.tile([P, TB], F32, name=f"psq{j}_{tb}",
                                    tag=f"psq{j}") for j in range(NHL)]
                    psk = ps1.tile([P, TB], F32, name=f"psk_{tb}", tag="psk")
                    psv = ps1.tile([P, TB], F32, name=f"psv_{tb}", tag="psv")
                    for a in range(NA):
                        xt_t = xp.tile([P, TB], DT_PROJ, name=f"x_{tb}_{a}",
                                       tag="xt")
                        nc.sync.dma_start(
                            out=xt_t,
                            in_=xt[a * P:(a + 1) * P, tb * TB:(tb + 1) * TB])
                        st, sp = (a == 0), (a == NA - 1)
                        nc.tensor.matmul(psk, wk_sb[:, a * DK:(a + 1) * DK],
                                         xt_t, start=st, stop=sp)
                        nc.tensor.matmul(psv, wv_sb[:, a * DK:(a + 1) * DK],
                                         xt_t, start=st, stop=sp)
                        for j in range(NHL):
                            nc.tensor.matmul(
                                psq[j],
                                wq_sb[:, a * FL + j * DK:a * FL + (j + 1) * DK],
                                xt_t, start=st, stop=sp)

                    # evict psum banks (one reader each, split ACT/DVE; K
                    # first since the next t-block's matmuls demand it first)
                    evs = []
                    plan = [(psk, k_scr, 0, nc.scalar),
                            (psq[0], q_scr, 0, nc.vector),
                            (psq[1], q_scr, P, nc.scalar),
                            (psq[2], q_scr, 2 * P, nc.vector),
                            (psq[3], q_scr, 3 * P, nc.scalar)]
                    for idx, (src, scr, r0, eng) in enumerate(plan):
                        qc = rp.tile([P, TB], F32, name=f"qc_{tb}_{idx}",
                                     tag="qc", bufs=7)
                        if eng is nc.scalar:
                            nc.scalar.copy(qc, src)
                        else:
                            nc.vector.tensor_copy(qc, src)
                        if idx == 0:
                            vb = rp.tile([P, TB], BF16, name=f"vb_{tb}",
                                         tag="vb", bufs=2)
                            nc.vector.tensor_copy(vb, psv)
                        evs.append((qc, scr, r0))
                    nc.scalar.dma_start(out=v_scr[bi][:, s0:s0 + TB], in_=vb)

                    # RoPE chains (SBUF only; eviction DMAs ride the ACT
                    # HWDGE ring so they never block the x-load stream)
                    for qc, scr, r0 in evs:
                        sw = rp.tile([P, TB], F32, name=f"sw_{tb}_{r0}",
                                     tag="sw", bufs=7)
                        nc.scalar.dma_start(out=sw[0:64, :], in_=qc[64:128, :])
                        nc.scalar.dma_start(out=sw[64:128, :], in_=qc[0:64, :])
                        nc.vector.tensor_mul(qc, qc, cos_sb[:, s0:s0 + TB])
                        nc.vector.tensor_mul(sw, sw, sin_sb[:, s0:s0 + TB])
                        qf = rp.tile([P, TB], DT_QK, name=f"qf_{tb}_{r0}",
                                     tag="qf", bufs=7)
                        nc.vector.tensor_add(qf, qc, sw)
                        nc.scalar.dma_start(
                            out=scr[bi][r0:r0 + P, s0:s0 + TB], in_=qf)

            # ------------- Phase 2 + interleaved output projection ----------
            with tc.tile_pool(name="ps2s", bufs=2, space="PSUM") as ps2s, \
                 tc.tile_pool(name="ps2u", bufs=2, space="PSUM") as ps2u, \
                 tc.tile_pool(name="ps2o", bufs=2, space="PSUM") as ps2o:
                wo_a, wo_b = {}, {}

                def load_wo(store, tag, ob):
                    wo_sb = p3w.tile([P, NHL, 512], DT_ATT,
                                     name=f"wo{tag}_{ob}_{len(store)}",
                                     tag=tag, bufs=2)
                    for j in range(NHL):
                        nc.sync.dma_start(
                            out=wo_sb[:, j, :],
                            in_=wot[j * P:(j + 1) * P,
                                    ob * 512:(ob + 1) * 512])
                    store[ob] = wo_sb

                ocnt = [0]

                def emit_otile(store, tag, bt, ob, ti, pre=None):
                    if ob not in store:
                        load_wo(store, tag, ob)
                    if pre is not None and pre not in store:
                        load_wo(store, tag, pre)
                    tt = bt * NTPB + ti
                    o_ps = ps2o.tile([P, 512], F32,
                                     name=f"o_{ocnt[0]}", tag="o")
                    ocnt[0] += 1
                    for j in range(NHL):
                        nc.tensor.matmul(
                            o_ps, attn_sb[bt][j][:, ti * P:(ti + 1) * P],
                            store[ob][:, j, :],
                            start=(j == 0), stop=(j == NHL - 1))
                    o_sb = p3o.tile([P, 512], F32,
                                    name=f"os_{bt}_{ob}_{ti}_{ocnt[0]}",
                                    tag="os")
                    nc.vector.tensor_copy(o_sb, o_ps)
                    nc.sync.dma_start(
                        out=out[tt * P:(tt + 1) * P, ob * 512:(ob + 1) * 512],
                        in_=o_sb)

                def emit_norm(b, h, qb, u_ps, d_ps):
                    rf_sb = p2.tile([1, QBS], F32, name=f"rf_{b}_{h}_{qb}",
                                    tag="rf", bufs=2)
                    nc.vector.reciprocal_approx_fast(rf_sb, d_ps)
                    rb_sb = p2.tile([P, QBS], F32, name=f"rs_{b}_{h}_{qb}",
                                    tag="rs", bufs=2)
                    nc.gpsimd.partition_broadcast(rb_sb, rf_sb)
                    nc.vector.tensor_mul(
                        attn_sb[b][h][:, qb * QBS:(qb + 1) * QBS],
                        u_ps, rb_sb)

                # batch-0 o-proj tiles drip-fed into batch-1's attention
                inter = [(0, ob, ti) for ob in range(NOB)
                         for ti in range(NTPB)]
                inter_pos = 0

                for b in range(B):
                    kb_sb = p2q.tile([P, S], DT_QK, name=f"kb_{b}", tag="kb")
                    nc.sync.dma_start(out=kb_sb, in_=k_scr[b][:, :])
                    vtk = p2q.tile([P, NKT, P], BF16, name=f"vt_{b}",
                                   tag="vtk")
                    nc.sync.dma_start_transpose(vtk, v_scr[b][:, :])
                    for qb in range(NQB):
                        nkt = 4 * qb + 4
                        for h in range(NHL):
                            qh_sb = p2q.tile([P, QBS], DT_QK,
                                             name=f"q_{b}_{h}_{qb}",
                                             tag="qh", bufs=3)
                            nc.sync.dma_start(
                                out=qh_sb,
                                in_=q_scr[b][h * P:(h + 1) * P,
                                             qb * QBS:(qb + 1) * QBS])
                            u_ps = ps2u.tile([P, QBS], F32,
                                             name=f"u_{b}_{h}_{qb}", tag="u")
                            d_ps = ps2u.tile([1, QBS], F32,
                                             name=f"d_{b}_{h}_{qb}", tag="d")

                            def emit_av(kt, e_sb, lo):
                                st, sp = (kt == 0), (kt == nkt - 1)
                                nc.tensor.matmul(u_ps[:, lo:], vtk[:, kt, :],
                                                 e_sb[:, lo:],
                                                 start=st, stop=sp,
                                                 skip_group_check=True)
                                nc.tensor.matmul(d_ps[:, lo:], oc_sb,
                                                 e_sb[:, lo:],
                                                 start=st, stop=sp,
                                                 skip_group_check=True)

                            av_fifo = []
                            for kt in range(nkt):
                                s_ps = ps2s.tile(
                                    [P, QBS], F32,
                                    name=f"s_{b}_{h}_{qb}_{kt}", tag="s")
                                m = kt - 4 * qb
                                lo = m * P if m > 0 else 0
                                nc.tensor.matmul(
                                    s_ps[:, lo:],
                                    kb_sb[:, kt * P:(kt + 1) * P],
                                    qh_sb[:, lo:],
                                    start=True, stop=True)
                                e_sb = p2e.tile(
                                    [P, QBS], BF16,
                                    name=f"e_{b}_{h}_{qb}_{kt}", tag="e")
                                nc.scalar.activation(e_sb[:, lo:],
                                                     s_ps[:, lo:], EXP,
                                                     scale=SCALE)
                                if m >= 0:
                                    nc.vector.tensor_mul(
                                        e_sb[:, m * P:(m + 1) * P],
                                        e_sb[:, m * P:(m + 1) * P],
                                        tri_sb)
                                if len(av_fifo) >= 2:
                                    emit_av(*av_fifo.pop(0))
                                av_fifo.append((kt, e_sb, lo))
                            for a0 in av_fifo:
                                emit_av(*a0)
                            emit_norm(b, h, qb, u_ps, d_ps)

                            # drip batch-0 o-proj into batch-1's attention
                            if b == 1:
                                for _ in range(8):
                                    if inter_pos < len(inter):
                                        bt0, ob0, ti0 = inter[inter_pos]
                                        emit_otile(
                                            wo_a, "woa", bt0, ob0, ti0,
                                            pre=(ob0 + 1 if ti0 == 4 and
                                                 ob0 + 1 < NOB else None))
                                        if ti0 == NTPB - 1:
                                            wo_a.pop(ob0, None)
                                        inter_pos += 1

                        # batch-1 rows for this q-block are now complete on
                        # all heads; project them while the next q-block's
                        # exp stream runs
                        if b == 1:
                            for ob in range(NOB):
                                for ti in range(4 * qb, 4 * qb + 4):
                                    emit_otile(wo_b, "wob", 1, ob, ti,
                                               pre=(ob + 1) % NOB
                                               if ti == 4 * qb else None)
                                wo_b.pop(ob, None)

    nc.compile()
    return nc


def _prep_inputs(hidden_states, Wq, Wk, Wv, Wo, cos, sin):
    hs = np.asarray(hidden_states, dtype=np.float32)
    Wq = np.asarray(Wq, dtype=np.float32)
    Wk = np.asarray(Wk, dtype=np.float32)
    Wv = np.asarray(Wv, dtype=np.float32)
    Wo = np.asarray(Wo, dtype=np.float32)
    cos = np.asarray(cos, dtype=np.float32)
    sin = np.asarray(sin, dtype=np.float32)

    xt = np.ascontiguousarray(hs.reshape(T, H).T).astype(NP_PROJ)
    cosT = np.ascontiguousarray(cos.T)
    sinT = np.ascontiguousarray(sin.T)
    sints = np.ascontiguousarray(
        np.concatenate([-sinT[:64], sinT[64:]], axis=0))
    kq = np.arange(P)
    trim = (kq[None, :] >= kq[:, None]).astype(ml_dtypes.bfloat16)
    onesc = np.ones((P, 1), dtype=ml_dtypes.bfloat16)

    in_maps = []
    for c in range(8):
        in_maps.append({
            "xt": xt,
            "wqt": np.ascontiguousarray(
                Wq[c * FL:(c + 1) * FL, :].T).astype(NP_PROJ),
            "wkt": np.ascontiguousarray(
                Wk[c * DK:(c + 1) * DK, :].T).astype(NP_PROJ),
            "wvt": np.ascontiguousarray(
                Wv[c * DK:(c + 1) * DK, :].T).astype(NP_PROJ),
            "wot": np.ascontiguousarray(
                Wo[:, c * FL:(c + 1) * FL].T).astype(NP_ATT),
            "cost": cosT,
            "sints": sints,
            "trimask": trim,
            "onesc": onesc,
        })
    return in_maps


def kernel(hidden_states, Wq, Wk, Wv, Wo, cos, sin, _run_kwargs=None):
    in_maps = _prep_inputs(hidden_states, Wq, Wk, Wv, Wo, cos, sin)
    if "nc" not in _NC_CACHE:
        _NC_CACHE["nc"] = build()
    nc = _NC_CACHE["nc"]
    kw = _run_kwargs or {}
    res = run_bass_kernel_spmd(nc, in_maps, core_ids=list(range(8)), **kw)
    acc = np.zeros((T, H), dtype=np.float64)
    for c in range(8):
        acc += np.asarray(res.results[c]["out"], dtype=np.float64)
    out = acc.astype(np.float32).reshape(B, S, H)
    if kw:
        _NC_CACHE["last_results"] = res
    return out


# revision 16
# speedup vs baseline: 1.0105x; 1.0105x over previous
"""Trainium2 Bass kernel for Llama GQA attention (B=2, S=2048, H=4096,
32 Q heads / 8 KV heads, head_dim 128, RoPE, causal).

Sharding: tensor-parallel by head across 8 cores. Core c owns Q heads
[4c..4c+3] and KV head c. Each core computes its Q/K/V projections,
RoPE, causal attention, and a partial output projection over its 512
attention features; the host sums the 8 partial outputs.

Device layout is feature-major ([feature, token]) throughout:
  - QKV proj:  Q'[f,t] (psum) = sum_h WqT[h,f].T @ xT[h,t]     (bf16)
  - RoPE:      q*cos + swap_halves(q)*sign*sin  (DVE + DMA swap)
  - scores:    S.T[k,q] = K'[d,k].T @ Q'[d,q]   (softmax over partition)
  - softmax:   exp on ACT (no max subtraction; scores are O(10)),
               denominator via ones-column matmul, fast reciprocal,
               K=1 broadcast matmul, normalize fused into psum evict
  - AV:        U[d,q] = Vtok[k,d].T @ E[k,q]    (bf16, causal-sliced)
  - out:       out[t,o] = attn'[f,t].T @ WoT[f,o]  (partial; host sums)

Batch-0's output projection is interleaved into batch-1's attention so
the PE chews o-proj matmuls while ACT runs the exp stream.
"""
import math
import numpy as np
import ml_dtypes

import concourse.bacc as bacc
import concourse.tile as tile
from concourse import mybir
from concourse.bass_utils import run_bass_kernel_spmd

F32 = mybir.dt.float32
F32R = mybir.dt.float32r
BF16 = mybir.dt.bfloat16

DT_PROJ = BF16
DT_QK = BF16
DT_ATT = BF16
NP_PROJ = ml_dtypes.bfloat16 if DT_PROJ == BF16 else np.float32
NP_ATT = ml_dtypes.bfloat16 if DT_ATT == BF16 else np.float32

P = 128
B, S, H = 2, 2048, 4096
T = B * S
DK = 128
NHL = 4
FL = NHL * DK
TB = 512
NTB = T // TB
NA = H // P
QBS = 512
NQB = S // QBS
NKT = S // P
SCALE = 1.0 / math.sqrt(DK)
NOB = H // 512
NTPB = S // P                # 16 output row tiles per batch

_NC_CACHE = {}


def build():
    nc = bacc.Bacc(None, target_bir_lowering=False)

    xt = nc.dram_tensor("xt", [H, T], DT_PROJ, kind="ExternalInput")
    wqt = nc.dram_tensor("wqt", [H, FL], DT_PROJ, kind="ExternalInput")
    wkt = nc.dram_tensor("wkt", [H, DK], DT_PROJ, kind="ExternalInput")
    wvt = nc.dram_tensor("wvt", [H, DK], DT_PROJ, kind="ExternalInput")
    wot = nc.dram_tensor("wot", [FL, H], DT_ATT, kind="ExternalInput")
    cost = nc.dram_tensor("cost", [P, S], F32, kind="ExternalInput")
    sints = nc.dram_tensor("sints", [P, S], F32, kind="ExternalInput")
    trimask = nc.dram_tensor("trimask", [P, P], BF16, kind="ExternalInput")
    identb = nc.dram_tensor("identb", [P, P], BF16, kind="ExternalInput")
    onesc = nc.dram_tensor("onesc", [P, 1], BF16, kind="ExternalInput")
    out = nc.dram_tensor("out", [T, H], F32, kind="ExternalOutput")

    EXP = mybir.ActivationFunctionType.Exp

    with nc.allow_low_precision(reason="attention compute dtypes are "
                                       "deliberately reduced"), \
         tile.TileContext(nc) as tc:
        with tc.tile_pool(name="const", bufs=1) as cp, \
             tc.tile_pool(name="dram", bufs=1, space="DRAM") as dp, \
             tc.tile_pool(name="attn", bufs=1) as ap, \
             tc.tile_pool(name="p2", bufs=1) as p2, \
             tc.tile_pool(name="p2q", bufs=2) as p2q, \
             tc.tile_pool(name="p2e", bufs=5) as p2e, \
             tc.tile_pool(name="p3w", bufs=2) as p3w, \
             tc.tile_pool(name="p3o", bufs=4) as p3o:
            cos_sb = cp.tile([P, S], F32)
            sin_sb = cp.tile([P, S], F32)
            ngt_sb = cp.tile([P, P], BF16)
            id_sb = cp.tile([P, P], BF16)
            oc_sb = cp.tile([P, 1], BF16)
            nc.sync.dma_start(out=cos_sb, in_=cost[:, :])
            nc.sync.dma_start(out=sin_sb, in_=sints[:, :])
            nc.sync.dma_start(out=ngt_sb, in_=trimask[:, :])
            nc.sync.dma_start(out=id_sb, in_=identb[:, :])
            nc.sync.dma_start(out=oc_sb, in_=onesc[:, :])

            attn_sb = [[ap.tile([P, S], DT_ATT, name=f"attn{b}_{h}")
                        for h in range(NHL)] for b in range(B)]
            q_scr = [dp.tile([FL, S], DT_QK, name=f"qscr{b}") for b in range(B)]
            k_scr = [dp.tile([DK, S], DT_QK, name=f"kscr{b}") for b in range(B)]
            v_scr = [dp.tile([DK, S], BF16, name=f"vscr{b}") for b in range(B)]

            # ---------------- Phase 1: QKV projection + RoPE ----------------
            with tc.tile_pool(name="wq", bufs=1) as wqp, \
                 tc.tile_pool(name="xp", bufs=10) as xp, \
                 tc.tile_pool(name="rp", bufs=1) as rp, \
                 tc.tile_pool(name="ps1", bufs=1, space="PSUM") as ps1:
                wq_sb = wqp.tile([P, NA * FL], DT_PROJ)
                wk_sb = wqp.tile([P, NA * DK], DT_PROJ)
                wv_sb = wqp.tile([P, NA * DK], DT_PROJ)
                for a in range(NA):
                    nc.sync.dma_start(out=wq_sb[:, a * FL:(a + 1) * FL],
                                      in_=wqt[a * P:(a + 1) * P, :])
                    nc.sync.dma_start(out=wk_sb[:, a * DK:(a + 1) * DK],
                                      in_=wkt[a * P:(a + 1) * P, :])
                    nc.sync.dma_start(out=wv_sb[:, a * DK:(a + 1) * DK],
                                      in_=wvt[a * P:(a + 1) * P, :])

                for tb in range(NTB):
                    bi = (tb * TB) // S
                    s0 = (tb * TB) % S
                    psq = [ps1.tile([P, TB], F32, name=f"psq{j}_{tb}",
                                    tag=f"psq{j}") for j in range(NHL)]
                    psk = ps1.tile([P, TB], F32, name=f"psk_{tb}", tag="psk")
                    psv = ps1.tile([P, TB], F32, name=f"psv_{tb}", tag="psv")
                    for a in range(NA):
                        xt_t = xp.tile([P, TB], DT_PROJ, name=f"x_{tb}_{a}",
                                       tag="xt")
                        nc.sync.dma_start(
                            out=xt_t,
                            in_=xt[a * P:(a + 1) * P, tb * TB:(tb + 1) * TB])
                        st, sp = (a == 0), (a == NA - 1)
                        nc.tensor.matmul(psk, wk_sb[:, a * DK:(a + 1) * DK],
                                         xt_t, start=st, stop=sp)
                        nc.tensor.matmul(psv, wv_sb[:, a * DK:(a + 1) * DK],
                                         xt_t, start=st, stop=sp)
                        for j in range(NHL):
                            nc.tensor.matmul(
                                psq[j],
                                wq_sb[:, a * FL + j * DK:a * FL + (j + 1) * DK],
                                xt_t, start=st, stop=sp)

                    # evict psum banks (one reader each, split ACT/DVE; K
                    # first since the next t-block's matmuls demand it first)
                    evs = []
                    plan = [(psk, k_scr, 0, nc.scalar),
                            (psq[0], q_scr, 0, nc.vector),
                            (psq[1], q_scr, P, nc.scalar),
                            (psq[2], q_scr, 2 * P, nc.vector),
                            (psq[3], q_scr, 3 * P, nc.scalar)]
                    for idx, (src, scr, r0, eng) in enumerate(plan):
                        qc = rp.tile([P, TB], F32, name=f"qc_{tb}_{idx}",
                                     tag="qc", bufs=7)
                        if eng is nc.scalar:
                            nc.scalar.copy(qc, src)
                        else:
                            nc.vector.tensor_copy(qc, src)
                        if idx == 0:
                            vb = rp.tile([P, TB], BF16, name=f"vb_{tb}",
                                         tag="vb", bufs=2)
                            nc.vector.tensor_copy(vb, psv)
                        evs.append((qc, scr, r0))
                    nc.scalar.dma_start(out=v_scr[bi][:, s0:s0 + TB], in_=vb)

                    # RoPE chains (SBUF only; eviction DMAs ride the ACT
                    # HWDGE ring so they never block the x-load stream)
                    for qc, scr, r0 in evs:
                        sw = rp.tile([P, TB], F32, name=f"sw_{tb}_{r0}",
                                     tag="sw", bufs=7)
                        nc.scalar.dma_start(out=sw[0:64, :], in_=qc[64:128, :])
                        nc.scalar.dma_start(out=sw[64:128, :], in_=qc[0:64, :])
                        nc.vector.tensor_mul(qc, qc, cos_sb[:, s0:s0 + TB])
                        nc.vector.tensor_mul(sw, sw, sin_sb[:, s0:s0 + TB])
                        qf = rp.tile([P, TB], DT_QK, name=f"qf_{tb}_{r0}",
                                     tag="qf", bufs=7)
                        nc.vector.tensor_add(qf, qc, sw)
                        nc.scalar.dma_start(
                            out=scr[bi][r0:r0 + P, s0:s0 + TB], in_=qf)

            # ------------- Phase 2 + interleaved output projection ----------
            with tc.tile_pool(name="ps2s", bufs=2, space="PSUM") as ps2s, \
                 tc.tile_pool(name="ps2u", bufs=2, space="PSUM") as ps2u, \
                 tc.tile_pool(name="ps2o", bufs=1, space="PSUM") as ps2o:
                wo_tiles = {}
                ocnt = [0]

                def load_wo(ob):
                    wo_sb = p3w.tile([P, NHL, 512], DT_ATT,
                                     name=f"wo_{ob}_{ocnt[0]}", tag="wo",
                                     bufs=2)
                    for j in range(NHL):
                        nc.sync.dma_start(
                            out=wo_sb[:, j, :],
                            in_=wot[j * P:(j + 1) * P,
                                    ob * 512:(ob + 1) * 512])
                    wo_tiles[ob] = wo_sb

                def emit_otile(bt, ob, ti):
                    if ob not in wo_tiles:
                        load_wo(ob)
                    if ti == 4 and ob + 1 < NOB and (ob + 1) not in wo_tiles:
                        load_wo(ob + 1)
                    tt = bt * NTPB + ti
                    o_ps = ps2o.tile([P, 512], F32, name=f"o_{ocnt[0]}",
                                     tag="o", bufs=2)
                    ocnt[0] += 1
                    for j in range(NHL):
                        nc.tensor.matmul(
                            o_ps, attn_sb[bt][j][:, ti * P:(ti + 1) * P],
                            wo_tiles[ob][:, j, :],
                            start=(j == 0), stop=(j == NHL - 1))
                    o_sb = p3o.tile([P, 512], F32, name=f"os_{ocnt[0]}",
                                    tag="os")
                    nc.vector.tensor_copy(o_sb, o_ps)
                    nc.sync.dma_start(
                        out=out[tt * P:(tt + 1) * P, ob * 512:(ob + 1) * 512],
                        in_=o_sb)
                    if ti == NTPB - 1:
                        wo_tiles.pop(ob, None)

                def emit_norm(b, h, qb, u_ps, d_ps):
                    rf_sb = p2.tile([1, QBS], F32, name=f"rf_{b}_{h}_{qb}",
                                    tag="rf", bufs=2)
                    nc.vector.reciprocal_approx_fast(rf_sb, d_ps)
                    rb_sb = p2.tile([P, QBS], F32, name=f"rs_{b}_{h}_{qb}",
                                    tag="rs", bufs=2)
                    nc.gpsimd.partition_broadcast(rb_sb, rf_sb)
                    nc.vector.tensor_mul(
                        attn_sb[b][h][:, qb * QBS:(qb + 1) * QBS],
                        u_ps, rb_sb)

                # batch-0 o-proj tiles drip-fed into batch-1's attention
                inter = [(0, ob, ti) for ob in range(NOB)
                         for ti in range(NTPB)]
                inter_pos = 0

                for b in range(B):
                    kb_sb = p2q.tile([P, S], DT_QK, name=f"kb_{b}", tag="kb")
                    nc.sync.dma_start(out=kb_sb, in_=k_scr[b][:, :])
                    vtk = p2q.tile([P, NKT, P], BF16, name=f"vt_{b}",
                                   tag="vtk")
                    nc.sync.dma_start_transpose(vtk, v_scr[b][:, :])
                    for h in range(NHL):
                        qh_sb = p2q.tile([P, S], DT_QK, name=f"q_{b}_{h}",
                                         tag="qh")
                        nc.sync.dma_start(out=qh_sb,
                                          in_=q_scr[b][h * P:(h + 1) * P, :])
                        for qb in range(NQB):
                            nkt = 4 * qb + 4
                            u_ps = ps2u.tile([P, QBS], F32,
                                             name=f"u_{b}_{h}_{qb}", tag="u")
                            d_ps = ps2u.tile([1, QBS], F32,
                                             name=f"d_{b}_{h}_{qb}", tag="d")

                            def emit_av(kt, e_sb, lo):
                                st, sp = (kt == 0), (kt == nkt - 1)
                                nc.tensor.matmul(u_ps[:, lo:], vtk[:, kt, :],
                                                 e_sb[:, lo:],
                                                 start=st, stop=sp,
                                                 skip_group_check=True)
                                nc.tensor.matmul(d_ps[:, lo:], oc_sb,
                                                 e_sb[:, lo:],
                                                 start=st, stop=sp,
                                                 skip_group_check=True)

                            av_fifo = []
                            for kt in range(nkt):
                                s_ps = ps2s.tile(
                                    [P, QBS], F32,
                                    name=f"s_{b}_{h}_{qb}_{kt}", tag="s")
                                m = kt - 4 * qb
                                lo = m * P if m > 0 else 0
                                nc.tensor.matmul(
                                    s_ps[:, lo:],
                                    kb_sb[:, kt * P:(kt + 1) * P],
                                    qh_sb[:, qb * QBS + lo:(qb + 1) * QBS],
                                    start=True, stop=False,
                                    skip_group_check=True)
                                if m >= 0:
                                    # causal mask: accumulate -1e30 upper
                                    # triangle on the diagonal chunk (PE,
                                    # keeps the exp->AV chain off the DVE)
                                    nc.tensor.matmul(
                                        s_ps[:, m * P:(m + 1) * P],
                                        id_sb, ngt_sb,
                                        start=False, stop=True,
                                        skip_group_check=True)
                                e_sb = p2e.tile(
                                    [P, QBS], BF16,
                                    name=f"e_{b}_{h}_{qb}_{kt}", tag="e")
                                nc.scalar.activation(e_sb[:, lo:],
                                                     s_ps[:, lo:], EXP,
                                                     scale=SCALE)
                                if len(av_fifo) >= 2:
                                    emit_av(*av_fifo.pop(0))
                                av_fifo.append((kt, e_sb, lo))
                                # drip one batch-0 o-proj tile per k-tile
                                if b == 1 and inter_pos < len(inter):
                                    emit_otile(*inter[inter_pos])
                                    inter_pos += 1
                            for a0 in av_fifo:
                                emit_av(*a0)
                            emit_norm(b, h, qb, u_ps, d_ps)
                while inter_pos < len(inter):
                    emit_otile(*inter[inter_pos])
                    inter_pos += 1
                wo_tiles.clear()
                for ob in range(NOB):
                    for ti in range(NTPB):
                        emit_otile(1, ob, ti)

    nc.compile()
    return nc


def _prep_inputs(hidden_states, Wq, Wk, Wv, Wo, cos, sin):
    hs = np.asarray(hidden_states, dtype=np.float32)
    Wq = np.asarray(Wq, dtype=np.float32)
    Wk = np.asarray(Wk, dtype=np.float32)
    Wv = np.asarray(Wv, dtype=np.float32)
    Wo = np.asarray(Wo, dtype=np.float32)
    cos = np.asarray(cos, dtype=np.float32)
    sin = np.asarray(sin, dtype=np.float32)

    xt = np.ascontiguousarray(hs.reshape(T, H).T).astype(NP_PROJ)
    cosT = np.ascontiguousarray(cos.T)
    sinT = np.ascontiguousarray(sin.T)
    sints = np.ascontiguousarray(
        np.concatenate([-sinT[:64], sinT[64:]], axis=0))
    kq = np.arange(P)
    trim = np.where(kq[None, :] < kq[:, None], -1e30, 0.0).astype(
        ml_dtypes.bfloat16)
    ident = np.eye(P, dtype=ml_dtypes.bfloat16)
    onesc = np.ones((P, 1), dtype=ml_dtypes.bfloat16)

    in_maps = []
    for c in range(8):
        in_maps.append({
            "xt": xt,
            "wqt": np.ascontiguousarray(
                Wq[c * FL:(c + 1) * FL, :].T).astype(NP_PROJ),
            "wkt": np.ascontiguousarray(
                Wk[c * DK:(c + 1) * DK, :].T).astype(NP_PROJ),
            "wvt": np.ascontiguousarray(
                Wv[c * DK:(c + 1) * DK, :].T).astype(NP_PROJ),
            "wot": np.ascontiguousarray(
                Wo[:, c * FL:(c + 1) * FL].T).astype(NP_ATT),
            "cost": cosT,
            "sints": sints,
            "trimask": trim,
            "identb": ident,
            "onesc": onesc,
        })
    return in_maps


def kernel(hidden_states, Wq, Wk, Wv, Wo, cos, sin, _run_kwargs=None):
    in_maps = _prep_inputs(hidden_states, Wq, Wk, Wv, Wo, cos, sin)
    if "nc" not in _NC_CACHE:
        _NC_CACHE["nc"] = build()
    nc = _NC_CACHE["nc"]
    kw = _run_kwargs or {}
    res = run_bass_kernel_spmd(nc, in_maps, core_ids=list(range(8)), **kw)
    acc = np.zeros((T, H), dtype=np.float64)
    for c in range(8):
        acc += np.asarray(res.results[c]["out"], dtype=np.float64)
    out = acc.astype(np.float32).reshape(B, S, H)
    if kw:
        _NC_CACHE["last_results"] = res
    return out
